# revision 11
# baseline (speedup 1.0000x reference)
"""Trainium2 Bass kernel for HardQuadRadiusTripletLoss.

Computes: per-keypoint dense correlation (2048x256 @ 256x3600 per image),
geometric radius masking (cells whose center is within 8px of the warped
keypoint), top-4 hard negatives, positive-cell similarity, and the
squared-hinge triplet loss reduced to a scalar.

Sharding: data-parallel over batch B=8 -> one image per NeuronCore.

Device pipeline per 128-keypoint tile (16 tiles/core), per 450-col chunk (8):
  PE  : d2m64 = [dy^2|dx^2|1]^T @ bpat      (f32r matmul -> dist2 - 64 in PSUM)
  ACT : u = relu(-K*(d2m64))                (K=2^20; f32r out; 0 outside mask)
  PE  : sim  = kp1_descT.T @ desc2          (f32r, 2 k-passes, PSUM)
        sim += (-I) @ u                     (neg-identity matmul applies mask)
  DVE : chunk top-8 = vector.max(sim_psum)  -> merge buffer
Per tile: DVE merge max over 8x8 chunk maxima -> top-8; indirect row-gather of
desc2T[flat_idx] + fused dot (scalar_tensor_tensor accum) -> positive sim.
Host: input transposes / coordinate prep, final relu(neg-pos+1)^2 mean.
"""

import sys

if "/opt/trn_rl_repo" not in sys.path:
    sys.path.insert(0, "/opt/trn_rl_repo")

import numpy as np

B, N, C, H, W = 8, 2048, 256, 60, 60
HW = H * W            # 3600
GRID = 8.0
NTILE = N // 128      # 16
NCHUNK = 8
CH = HW // NCHUNK     # 450
KPEN = float(2 ** 20)

_NC_CACHE = {}


def _build_nc():
    from concourse import bacc, mybir, bass
    import concourse.tile as tile

    nc = bacc.Bacc("TRN2", target_bir_lowering=False, debug=False)
    f32 = mybir.dt.float32
    f32r = mybir.dt.float32r
    i32 = mybir.dt.int32
    Alu = mybir.AluOpType
    Act = mybir.ActivationFunctionType

    d_desc2f = nc.dram_tensor("desc2f", (C, HW), f32, kind="ExternalInput").ap()
    d_desc2T = nc.dram_tensor("desc2T", (HW, C), f32, kind="ExternalInput").ap()
    d_kpT = nc.dram_tensor("kpT", (C, N), f32, kind="ExternalInput").ap()
    d_kpn = nc.dram_tensor("kpn", (N, C), f32, kind="ExternalInput").ap()
    d_dyxT = nc.dram_tensor("dyxT", (121, N), f32, kind="ExternalInput").ap()
    d_bpat = nc.dram_tensor("bpat", (121, HW), f32, kind="ExternalInput").ap()
    d_negid = nc.dram_tensor("negid", (128, 128), f32, kind="ExternalInput").ap()
    d_fidx = nc.dram_tensor("fidx", (N, 1), i32, kind="ExternalInput").ap()
    d_top8 = nc.dram_tensor("top8", (N, 8), f32, kind="ExternalOutput").ap()
    d_pos = nc.dram_tensor("pos", (N, 1), f32, kind="ExternalOutput").ap()

    with tile.TileContext(nc) as tc:
        with (
            tc.tile_pool(name="pers", bufs=1) as pers,
            tc.tile_pool(name="stage", bufs=2) as stage,
            tc.tile_pool(name="upool", bufs=3) as upool,
            tc.tile_pool(name="work", bufs=3) as work,
            tc.tile_pool(name="ps_d", bufs=2, space="PSUM") as ps_d,
            tc.tile_pool(name="ps_s", bufs=4, space="PSUM") as ps_s,
        ):
            # Persistent f32r operands: DMA load then the mandatory
            # f32r-rounding cast (DVE).
            def load_r(nm, dram_ap, shape):
                st = stage.tile(list(shape), f32, tag="stage")
                nc.sync.dma_start(st[:], dram_ap)
                tr = pers.tile(list(shape), f32r, tag=nm)
                nc.vector.tensor_copy(tr[:], st[:])
                return tr

            dyxT = load_r("dyxT", d_dyxT[:], (121, N))
            bp = load_r("bp", d_bpat[:], (121, HW))
            kpT0 = load_r("kpT0", d_kpT[0:128, :], (128, N))
            kpT1 = load_r("kpT1", d_kpT[128:256, :], (128, N))
            rhs0 = load_r("rhs0", d_desc2f[0:128, :], (128, HW))
            rhs1 = load_r("rhs1", d_desc2f[128:256, :], (128, HW))
            negid = load_r("negid", d_negid[:], (128, 128))

            for t in range(NTILE):
                ns = slice(t * 128, (t + 1) * 128)

                # ---- positive similarity path (exact fp32) ----
                kpn_t = work.tile([128, C], f32, tag="kpn")
                nc.sync.dma_start(kpn_t[:], d_kpn[ns, :])
                fidx_t = work.tile([128, 1], i32, tag="fidx")
                nc.sync.dma_start(fidx_t[:], d_fidx[ns, :])
                posd_t = work.tile([128, C], f32, tag="posd")
                nc.gpsimd.indirect_dma_start(
                    out=posd_t[:],
                    out_offset=None,
                    in_=d_desc2T[:],
                    in_offset=bass.IndirectOffsetOnAxis(ap=fidx_t[:, :1], axis=0),
                )
                junk_t = work.tile([128, C], f32, tag="junk")
                pos_t = work.tile([128, 1], f32, tag="pos")
                nc.vector.scalar_tensor_tensor(
                    out=junk_t[:],
                    in0=posd_t[:],
                    scalar=1.0,
                    in1=kpn_t[:],
                    op0=Alu.mult,
                    op1=Alu.mult,
                    accum_out=pos_t[:],
                )
                nc.sync.dma_start(d_pos[ns, :], pos_t[:])

                # ---- dense correlation + mask + chunkwise top8 ----
                m64 = work.tile([128, 64], f32, tag="m64")
                for c in range(NCHUNK):
                    cs = slice(c * CH, (c + 1) * CH)
                    d2 = ps_d.tile([128, CH], f32, tag="d2")
                    nc.tensor.matmul(
                        out=d2[:], lhsT=dyxT[:, ns], rhs=bp[:, cs],
                        start=True, stop=True,
                    )
                    u = upool.tile([128, CH], f32r, tag="u")
                    nc.scalar.activation(
                        out=u[:], in_=d2[:], func=Act.Relu, scale=-KPEN,
                    )
                    sm = ps_s.tile([128, CH], f32, tag="sm")
                    nc.tensor.matmul(
                        out=sm[:], lhsT=kpT0[:, ns], rhs=rhs0[:, cs],
                        start=True, stop=False,
                    )
                    nc.tensor.matmul(
                        out=sm[:], lhsT=kpT1[:, ns], rhs=rhs1[:, cs],
                        start=False, stop=False,
                    )
                    nc.tensor.matmul(
                        out=sm[:], lhsT=negid[:], rhs=u[:],
                        start=False, stop=True,
                    )
                    nc.vector.max(out=m64[:, c * 8:(c + 1) * 8], in_=sm[:])

                top8_t = work.tile([128, 8], f32, tag="top8")
                nc.vector.max(out=top8_t[:], in_=m64[:])
                nc.sync.dma_start(d_top8[ns, :], top8_t[:])

    nc.compile()
    return nc


def get_nc():
    if "nc" not in _NC_CACHE:
        _NC_CACHE["nc"] = _build_nc()
    return _NC_CACHE["nc"]


def make_in_maps(w_kp1, kp1_desc, desc2):
    yc = ((np.arange(H, dtype=np.float32) + np.float32(0.5)) * np.float32(GRID))
    bpat = np.zeros((121, HW), np.float32)
    for h in range(H):
        bpat[h, h * W:(h + 1) * W] = 1.0
    for w in range(W):
        bpat[60 + w, w::W] = 1.0
    bpat[120, :] = -64.0
    negid = -np.eye(128, dtype=np.float32)

    in_maps = []
    for b in range(B):
        wb = np.asarray(w_kp1[b], dtype=np.float32)
        cy = np.clip(np.floor(wb[:, 0] / np.float32(GRID)).astype(np.int32), 0, H - 1)
        cx = np.clip(np.floor(wb[:, 1] / np.float32(GRID)).astype(np.int32), 0, W - 1)
        fidx = (cy * W + cx).astype(np.int32).reshape(N, 1)
        dy = wb[:, 0:1] - yc[None, :]
        dx = wb[:, 1:2] - yc[None, :]
        dyxT = np.empty((121, N), np.float32)
        dyxT[0:60] = (dy * dy).T
        dyxT[60:120] = (dx * dx).T
        dyxT[120] = 1.0
        kpd = np.ascontiguousarray(np.asarray(kp1_desc[b], dtype=np.float32))
        d2f = np.ascontiguousarray(np.asarray(desc2[b], dtype=np.float32).reshape(C, HW))
        in_maps.append({
            "desc2f": d2f,
            "desc2T": np.ascontiguousarray(d2f.T),
            "kpT": np.ascontiguousarray(kpd.T),
            "kpn": kpd,
            "dyxT": np.ascontiguousarray(dyxT),
            "bpat": bpat,
            "negid": negid,
            "fidx": fidx,
        })
    return in_maps


def finish_loss(results):
    total = 0.0
    for b in range(B):
        out = results[b]
        neg4 = out["top8"][:, :4].astype(np.float64)
        pos = out["pos"].astype(np.float64)
        t = np.maximum(neg4 - pos + 1.0, 0.0)
        total += float((t * t).sum())
    return np.asarray(np.float32(total / (B * N * 4)))


def kernel(kp1, w_kp1, kp1_desc, desc2, homo12):
    from concourse.bass_utils import run_bass_kernel_spmd

    nc = get_nc()
    in_maps = make_in_maps(w_kp1, kp1_desc, desc2)
    res = run_bass_kernel_spmd(nc, in_maps, core_ids=list(range(B)))
    return finish_loss(res.results)
